# revision 60
# baseline (speedup 1.0000x reference)
"""BiMPM matching kernel for Trainium2 (Bass/Tile), 8-core data-parallel.

Strategy: batch B=8 is sharded one element per NeuronCore. Each core runs the
full BiMPM forward for its (L=128, D=512) pair of contexts:
  - pairwise cosine via PE matmuls on row-normalized contexts
  - full / attentive / max-attentive matching via small matmuls + fused DVE ops
  - maxpool matching (16 perspectives) via per-perspective PE matmuls
  - the (L1,L2,D) masked "max attentive" tensors via a 128-iteration
    select-broadcast matmul (identity-column x context) + fused
    scalar_tensor_tensor mul+max accumulation on the Vector engine
Weights are tiny (16,512) and replicated to every core.

Self-contained: hardcodes shapes B=8, L1=L2=128, D=512, P=16.
"""

import numpy as np

import concourse.bass as bass
import concourse.mybir as mybir
import concourse.tile as tile
from concourse.bass_utils import run_bass_kernel_spmd
from concourse.vector_clock import ScopedClock

f32 = mybir.dt.float32
f16 = mybir.dt.float16
ALU = mybir.AluOpType
AFT = mybir.ActivationFunctionType
AX = mybir.AxisListType

B, L, D, P = 8, 128, 512, 16
NCH = D // 128  # 4 d-chunks
NEG = -1.0e30
EPS_CNT = 1.0e-8  # matches reference EPS for count clamping
EPS_N = 1.0e-6    # per-factor norm clamp (product >= 1e-12 never binds on this data)

# PRECISE=True: everything fp32 (rel err ~6e-6, slower).
# PRECISE=False: fp16 data path for the attentive-max loops and the maxpool
# matmuls (rel err ~1e-3, ~2x faster). Mask fill uses -60000 (fp16-finite;
# only ever compared against, never emitted: every row has >=1 valid entry).
PRECISE = False
OFFBIG = 60000.0

# Tunables for the attentive-max pipeline (swept against TimelineSim).
PSP_BUFS = 6   # PSUM product tiles in flight ([128,512] each, 1 bank)
PSA_BUFS = 1
PSB_BUFS = 1
DROUTE = (0, 3, 6, 9, 12)  # t%16 values using the DVE-direct-from-PSUM route
BM_BATCH = False  # batch maxpool means into one PSUM tile (costs a bank)
S2PER = 2      # side-2 att steps emitted per loop iteration
S1NUM, S1DEN = 1, 2  # side-1 allowed up to S1NUM/S1DEN of h1 until side-2 done
ROFF = {"2": 0, "1": 1}  # per-side phase offset of the D-route pattern
BCH_BUFS = 18
VSQT_DVE = True  # square vT on DVE (TT mult) instead of ACT

# ---------------------------------------------------------------------------
# Workarounds: this walrus build accepts only ONE sync-wait per instruction.
# ---------------------------------------------------------------------------

def _drain_and_barrier_split(self, tick_clock, wait_clock):
    drain_inst = self.nc.sync.drain()
    wait_clock.add_sem_waits(
        drain_inst.ins, ScopedClock({None: tick_clock.global_clock})
    )
    si = drain_inst.ins.sync_info
    if si is not None and si.on_wait and len(si.on_wait) > 1:
        extra = list(si.on_wait[1:])
        del si.on_wait[1:]
        for w in extra:
            d2 = self.nc.sync.drain()
            if d2.ins.sync_info is None:
                d2.ins.sync_info = mybir.SyncInfo(on_wait=[], on_update=[])
            d2.ins.sync_info.on_wait.append(w)
    self.nc.all_engine_barrier()
    assert self.sems is not None
    popped = self.nc._tile_sem_poison_stack.pop()
    assert popped is self._sem_poison
    self.nc.clear_and_free_semaphores(list(self.sems.allocated().values()))


tile.TileContext._drain_and_barrier = _drain_and_barrier_split


def _split_multi_waits(nc):
    """Hoist extra sync-waits onto injected same-engine Drains placed before
    the owning instruction (serial on one engine == wait-all)."""
    n = 0
    for fn in nc.m.functions:
        for blk in fn.blocks:
            new = []
            for ins in blk.instructions:
                si = ins.sync_info
                if si is not None and si.on_wait and len(si.on_wait) > 1:
                    extra = list(si.on_wait[:-1])
                    keep = [si.on_wait[-1]]
                    for w in extra:
                        new.append(
                            mybir.InstDrain(
                                name=f"waitsplit-{n}",
                                engine=ins.engine,
                                is_reset_sema=False,
                                sync_info=mybir.SyncInfo(on_wait=[w], on_update=[]),
                            )
                        )
                        n += 1
                    si.on_wait = keep
                new.append(ins)
            blk.instructions = new
    return n


# ---------------------------------------------------------------------------
# Kernel emission
# ---------------------------------------------------------------------------

def CH(k):
    return slice(k * 128, (k + 1) * 128)


def _emit(nc: bass.Bass, h1=64, h2=64):
    """h1/h2: pair-step counts for the attentive-max loops (side B / side A).
    Side A covers j in [0, 2*h2) of context-2 rows; rows >= len2 are masked
    by the -60000 offsets, so any h2 >= ceil(len2max/2) is exact."""
    ctx1_d = nc.dram_tensor("context_1", [L, D], f32, kind="ExternalInput")
    m1_d = nc.dram_tensor("mask_1", [1, L], f32, kind="ExternalInput")
    ctx2_d = nc.dram_tensor("context_2", [L, D], f32, kind="ExternalInput")
    m2_d = nc.dram_tensor("mask_2", [1, L], f32, kind="ExternalInput")
    wff_d = nc.dram_tensor("w_full_fwd", [P, D], f32, kind="ExternalInput")
    wbw_d = nc.dram_tensor("w_full_bwd", [P, D], f32, kind="ExternalInput")
    wmp_d = nc.dram_tensor("w_maxpool", [P, D], f32, kind="ExternalInput")
    wat_d = nc.dram_tensor("w_att", [P, D], f32, kind="ExternalInput")
    wma_d = nc.dram_tensor("w_max_att", [P, D], f32, kind="ExternalInput")
    idn_d = nc.dram_tensor("idn", [128, 128], f32, kind="ExternalInput")
    out_d = nc.dram_tensor("out", [L, 204], f32, kind="ExternalOutput")

    with tile.TileContext(nc) as tc:
        with tc.tile_pool(name="sb", bufs=1) as sb, \
             tc.tile_pool(name="sc", bufs=2) as sc, \
             tc.tile_pool(name="psA", bufs=PSA_BUFS, space="PSUM") as psA, \
             tc.tile_pool(name="psP", bufs=PSP_BUFS, space="PSUM") as psP, \
             tc.tile_pool(name="psB", bufs=PSB_BUFS, space="PSUM") as psB:

            def tA():
                return psA.tile([128, 512], f32, tag="a", name="psa")

            def tD(name="psd"):
                # small maxpool tiles, in the psB rotation
                return psB.tile([128, 128], f32, tag="b", name=name,
                                padded_shape=[128, 512])

            def tD1(name="psd1"):
                return psB.tile([128, 1], f32, tag="b", name=name,
                                padded_shape=[128, 512])

            def tB():
                return psB.tile([128, 512], f32, tag="b", name="psb")

            def scr512():
                return sc.tile([128, 512], f32, tag="scr512", name="scr512")

            EPS_N2 = EPS_N * EPS_N

            def rsqrt_clamped(shape, tagbase, src_ap):
                """1/max(sqrt(x), EPS_N) ~= 1/sqrt(x + EPS_N^2): exact for
                x >> EPS_N^2 and for x == 0; the clamp only ever binds on
                all-zero (masked) rows where the numerator is 0 anyway."""
                sq = sb.tile(shape, f32, tag=f"{tagbase}_sq", name="rsq_s")
                nc.scalar.activation(sq[:], src_ap, AFT.Sqrt,
                                     bias=epscol[0:shape[0], :])
                r = sb.tile(shape, f32, tag=f"{tagbase}_r", name="rsq_r")
                nc.vector.reciprocal(r[:], sq[:])
                return r

            # ---------------- constants + inputs ----------------
            m1row = sb.tile([1, 128], f32, tag="m1row")
            nc.sync.dma_start(m1row[:], m1_d[:])
            m2row = sb.tile([1, 128], f32, tag="m2row")
            nc.sync.dma_start(m2row[:], m2_d[:])
            ctx1 = sb.tile([128, 512], f32, tag="ctx1")
            nc.sync.dma_start(ctx1[:], ctx1_d[:])
            ctx2 = sb.tile([128, 512], f32, tag="ctx2")
            nc.sync.dma_start(ctx2[:], ctx2_d[:])
            idn = sb.tile([128, 128], f32, tag="idn")
            nc.sync.dma_start(idn[:], idn_d[:])
            ones1 = sb.tile([1, 128], f32, tag="ones1")
            nc.vector.memset(ones1[:], 1.0)
            one11 = sb.tile([1, 1], f32, tag="one11")
            nc.vector.memset(one11[:], 1.0)
            onescol = sb.tile([128, 1], f32, tag="onescol")
            nc.vector.memset(onescol[:], 1.0)
            epscol = sb.tile([128, 1], f32, tag="epscol")
            nc.vector.memset(epscol[:], EPS_N * EPS_N)
            w_all = sb.tile([80, 512], f32, tag="w_all", name="w_all")
            for wi, wd in enumerate((wff_d, wbw_d, wmp_d, wat_d, wma_d)):
                nc.sync.dma_start(w_all[16 * wi:16 * (wi + 1), :], wd[:])

            out12 = sb.tile([128, 204], f32, tag="out12")

            # ---------------- masks / columns ----------------
            def row_to_col(row, n=128):
                ps = psB.tile([128, 1], f32, tag="b", name="r2c",
                              padded_shape=[128, 512])
                nc.tensor.matmul(ps[:n, :], lhsT=row[:, 0:n], rhs=one11[:],
                                 start=True, stop=True)
                col = sb.tile([n, 1], f32, tag=f"col{nc.next_id()}", name="col")
                nc.vector.tensor_copy(col[:], ps[:n, :])
                return col

            m1col = row_to_col(m1row)
            m2col = row_to_col(m2row)

            c1 = sb.tile([128, 512], f32, tag="c1")
            nc.vector.tensor_scalar(c1[:], ctx1[:], m1col[:], None, ALU.mult)
            c2 = sb.tile([128, 512], f32, tag="c2")
            nc.vector.tensor_scalar(c2[:], ctx2[:], m2col[:], None, ALU.mult)

            # mask helpers
            def ts_new(shape, tag, in0, s1, s2, op0, op1=None):
                t = sb.tile(shape, f32, tag=tag, name=tag)
                if op1 is None:
                    nc.vector.tensor_scalar(t[:], in0[:], s1, None, op0)
                else:
                    nc.vector.tensor_scalar(t[:], in0[:], s1, s2, op0, op1)
                return t

            lp = f32 if PRECISE else f16
            invm1row = ts_new([1, 128], "invm1row", m1row, -1.0, 1.0, ALU.mult, ALU.add)
            invm2row = ts_new([1, 128], "invm2row", m2row, -1.0, 1.0, ALU.mult, ALU.add)
            offm1row = ts_new([1, 128], "offm1row", m1row, -1.0, 1.0e30, ALU.add, ALU.mult)
            offm2row = ts_new([1, 128], "offm2row", m2row, -1.0, 1.0e30, ALU.add, ALU.mult)

            # counts: rcnt = 1/max(sum(mask), EPS)
            def rcnt_of(mrow, tag):
                s = sb.tile([1, 1], f32, tag=f"cnt_{tag}", name="cnt")
                nc.vector.tensor_reduce(s[:], mrow[:], AX.X, ALU.add)
                sc_ = sb.tile([1, 1], f32, tag=f"cntc_{tag}", name="cntc")
                nc.vector.tensor_scalar(sc_[:], s[:], EPS_CNT, None, ALU.max)
                r = sb.tile([1, 1], f32, tag=f"rcnt_{tag}", name="rcnt")
                nc.vector.reciprocal(r[:], sc_[:])
                return r

            rcnt1 = rcnt_of(m1row, "1")
            rcnt2 = rcnt_of(m2row, "2")
            m1rowS = ts_new([1, 128], "m1rowS", m1row, rcnt1[:], None, ALU.mult)
            m2rowS = ts_new([1, 128], "m2rowS", m2row, rcnt2[:], None, ALU.mult)
            m1sd = row_to_col(m1rowS)  # mask/cnt as column, for PE mean-reduces
            m2sd = row_to_col(m2rowS)

            # broadcast rows across partitions (PE outer product), keep in SBUF
            def bcast_row(row, tag, act=False):
                ps = psB.tile([128, 128], f32, tag="b", name="bcr",
                              padded_shape=[128, 512])
                nc.tensor.matmul(ps[:], lhsT=ones1[:], rhs=row[:], start=True, stop=True)
                t = sb.tile([128, 128], f32, tag=tag, name=tag)
                if act:
                    nc.scalar.copy(t[:], ps[:])
                else:
                    nc.vector.tensor_copy(t[:], ps[:])
                return t


            # ---------------- norms + normalized contexts ----------------
            def normalize(cx, tag):
                nsq = sb.tile([128, 1], f32, tag=f"nsq_{tag}", name="nsq")
                nc.scalar.activation(scr512()[:], cx[:], AFT.Square, accum_out=nsq[:])
                rn = rsqrt_clamped([128, 1], f"rn_{tag}", nsq[:])
                cn = sb.tile([128, 512], f32, tag=f"cn_{tag}", name="cn")
                nc.vector.tensor_scalar(cn[:], cx[:], rn[:], None, ALU.mult)
                return cn, rn

            cn1, rn1 = normalize(c1, "1")
            cn2, rn2 = normalize(c2, "2")

            # transposed normalized contexts: cnXT[:, CH(k)] = cnX[:, CH(k)].T
            def transpose512(src, tag, engine_copy="v", dt=f32):
                ps = tA()
                for k in range(NCH):
                    nc.tensor.transpose(ps[:, CH(k)], src[:, CH(k)], idn[:])
                t = sb.tile([128, 512], dt, tag=tag, name=tag)
                if engine_copy == "v":
                    nc.vector.tensor_copy(t[:], ps[:])
                else:
                    nc.scalar.copy(t[:], ps[:])
                return t

            c1T = transpose512(cn1, "c1T", "s")
            c2T = transpose512(cn2, "c2T", "s")
            c1sqT = sb.tile([128, 512], f32, tag="c1sqT")
            nc.scalar.square(c1sqT[:], c1T[:])
            c2sqT = sb.tile([128, 512], f32, tag="c2sqT")
            nc.scalar.square(c2sqT[:], c2T[:])

            # ---------------- cosine ----------------
            cos_ps = psB.tile([128, 128], f32, tag="b", name="cos_ps",
                              padded_shape=[128, 512])
            for k in range(NCH):
                nc.tensor.matmul(cos_ps[:], lhsT=c1T[:, CH(k)], rhs=c2T[:, CH(k)],
                                 start=(k == 0), stop=(k == NCH - 1))
            cos = sb.tile([128, 128], f32, tag="cos")
            nc.vector.tensor_copy(cos[:], cos_ps[:])
            cosT_ps = psB.tile([128, 128], f32, tag="b", name="cosT_ps",
                               padded_shape=[128, 512])
            nc.tensor.transpose(cosT_ps[:], cos[:], idn[:])
            cosT = sb.tile([128, 128], f32, tag="cosT")
            nc.vector.tensor_copy(cosT[:], cosT_ps[:])

            # att-loop scalar sources: +1 at invalid columns (free dim)
            inv2_ps = psB.tile([128, 128], f32, tag="b", name="inv2_ps",
                               padded_shape=[128, 512])
            nc.tensor.matmul(inv2_ps[:], lhsT=ones1[:], rhs=invm2row[:],
                             start=True, stop=True)
            cosM = sb.tile([128, 128], f32, tag="cosM")
            nc.vector.tensor_tensor(cosM[:], cos[:], inv2_ps[:], ALU.add)
            inv1_ps = psB.tile([128, 128], f32, tag="b", name="inv1_ps",
                               padded_shape=[128, 512])
            nc.tensor.matmul(inv1_ps[:], lhsT=ones1[:], rhs=invm1row[:],
                             start=True, stop=True)
            cosMT = sb.tile([128, 128], f32, tag="cosMT")
            nc.vector.tensor_tensor(cosMT[:], cosT[:], inv1_ps[:], ALU.add)

            # att-loop data sources: big negative at invalid rows (partition dim)
            if PRECISE:
                offb1col = ts_new([128, 1], "offb1col", m1col, -1.0, 1.0e30,
                                  ALU.add, ALU.mult)
                offb2col = ts_new([128, 1], "offb2col", m2col, -1.0, 1.0e30,
                                  ALU.add, ALU.mult)
            else:
                offb1col = ts_new([128, 1], "offb1col", m1col, -1.0, OFFBIG,
                                  ALU.add, ALU.mult)
                offb2col = ts_new([128, 1], "offb2col", m2col, -1.0, OFFBIG,
                                  ALU.add, ALU.mult)
            c1M = sb.tile([128, 512], lp, tag="c1M")
            nc.vector.tensor_scalar(c1M[:], ctx1[:], m1col[:], offb1col[:],
                                    ALU.mult, ALU.add)
            c2M = sb.tile([128, 512], lp, tag="c2M")
            nc.vector.tensor_scalar(c2M[:], ctx2[:], m2col[:], offb2col[:],
                                    ALU.mult, ALU.add)
            if PRECISE:
                idnL = idn
            else:
                idnL = sb.tile([128, 128], f16, tag="idnL")
                nc.vector.tensor_copy(idnL[:], idn[:])

            # ---------------- cos_max / cos_mean (out cols 0,1) ----------------
            def masked_mean_pe(srcT, msd, out_col, name):
                # out[i] = sum_j src[i, j] * msd[j] as a PE contraction
                ps = psB.tile([128, 1], f32, tag="b", name=name,
                              padded_shape=[128, 512])
                nc.tensor.matmul(ps[:], lhsT=srcT[:], rhs=msd[:],
                                 start=True, stop=True)
                nc.vector.tensor_copy(out_col, ps[:])

            # masked row-max: accumulate the big-negative mask fill onto the
            # cosine PSUM tiles (after the clean copies were taken), reduce
            nc.tensor.matmul(cos_ps[:], lhsT=ones1[:], rhs=offm2row[:],
                             start=False, stop=True, skip_group_check=True)
            nc.vector.tensor_reduce(out12[:, 0:1], cos_ps[:], AX.X, ALU.max)
            nc.tensor.matmul(cosT_ps[:], lhsT=ones1[:], rhs=offm1row[:],
                             start=False, stop=True, skip_group_check=True)
            nc.vector.tensor_reduce(out12[:, 102:103], cosT_ps[:], AX.X,
                                    ALU.max)
            masked_mean_pe(cosT, m2sd, out12[:, 1:2], "cmean1")
            masked_mean_pe(cos, m1sd, out12[:, 103:104], "cmean2")

            # ---------------- per-weight prep ----------------
            # wall_sqT: [128, 320], chunk k at cols [80k, 80k+80) holds the
            # transposed squares of all 5 (16,512) weight tiles for d-chunk k.
            W_IDX = {"ff": 0, "bw": 1, "mp": 2, "at": 3, "ma": 4}
            psW5 = psB.tile([128, 320], f32, tag="b", name="psW5",
                            padded_shape=[128, 512])
            for k in range(NCH):
                nc.tensor.transpose(psW5[:, 80 * k:80 * k + 80],
                                    w_all[:, CH(k)], idn[0:80, 0:80])
            wall_sqT = sb.tile([128, 320], f32, tag="wall_sqT")
            nc.scalar.square(wall_sqT[:], psW5[:])

            wall_sqT16 = sb.tile([128, 320], f16, tag="wall_sqT16")
            nc.vector.tensor_copy(wall_sqT16[:], wall_sqT[:])
            onescol16 = sb.tile([128, 1], f16, tag="onescol16")
            nc.vector.memset(onescol16[:], 1.0)

            def wsqTs(wname, k, a=0, b=16):
                base = 80 * k + 16 * W_IDX[wname]
                return wall_sqT[:, base + a:base + b]

            def wsqTs16(wname, k, a=0, b=16):
                base = 80 * k + 16 * W_IDX[wname]
                return wall_sqT16[:, base + a:base + b]

            rnp1 = {}
            rnp2 = {}

            def rnp_of(csqT, wname, side):
                ps = psB.tile([128, P], f32, tag="b", name="psnp",
                              padded_shape=[128, 512])
                for k in range(NCH):
                    nc.tensor.matmul(ps[:], lhsT=csqT[:, CH(k)],
                                     rhs=wsqTs(wname, k),
                                     start=(k == 0), stop=(k == NCH - 1))
                return rsqrt_clamped([128, P], f"rnp_{wname}{side}", ps[:])

            for wname in ("ff", "bw", "mp", "at", "ma"):
                rnp1[wname] = rnp_of(c1sqT, wname, "1")
                rnp2[wname] = rnp_of(c2sqT, wname, "2")

            # ---------------- maxpool matching ----------------
            if PRECISE:
                c1TL, c2TL = c1T, c2T
            else:
                c1TL = sb.tile([128, 512], f16, tag="c1TL")
                nc.vector.tensor_copy(c1TL[:], c1T[:])
                c2TL = sb.tile([128, 512], f16, tag="c2TL")
                nc.vector.tensor_copy(c2TL[:], c2T[:])
            # ---------------- full matching (last/first rows) ----------------
            def onehot_last(mrow, tag):
                oh = sb.tile([1, 128], f32, tag=f"oh_{tag}", name="oh")
                nc.vector.tensor_sub(oh[:, 0:127], mrow[:, 0:127], mrow[:, 1:128])
                nc.vector.tensor_copy(oh[:, 127:128], mrow[:, 127:128])
                return oh

            def extract_row(coltile, src, tag):
                # (1,512) = coltile.T @ src
                ps = psA.tile([1, 512], f32, tag="a", name="exr",
                              padded_shape=[128, 512])
                nc.tensor.matmul(ps[:], lhsT=coltile[:], rhs=src[:],
                                 start=True, stop=True)
                t = sb.tile([1, 512], f32, tag=f"row_{tag}", name="rowx")
                nc.vector.tensor_copy(t[:], ps[:])
                return t

            oh2 = onehot_last(m2row, "2")
            oh2c = row_to_col(oh2)
            c2last = extract_row(oh2c, c2, "c2l")
            oh1 = onehot_last(m1row, "1")
            oh1c = row_to_col(oh1)
            c1last = extract_row(oh1c, c1, "c1l")

            def row_match(rowvec, wname, rn_self_p, cTSelf, base):
                """rowvec: (1,512) raw matching row. Emits s (base) and 16
                multi-perspective cols (base+1..base+16). Cosines are
                scale-invariant in rowvec, so no normalization of it is
                needed except for the s-feature denominator."""
                u = f"rm{base}"
                # raw row as column chunks (128,4), and its square
                psL = psB.tile([128, NCH], f32, tag="b", name="psL",
                               padded_shape=[128, 512])
                for k in range(NCH):
                    nc.tensor.matmul(psL[:, k:k + 1], lhsT=rowvec[:, CH(k)],
                                     rhs=one11[:], start=True, stop=True)
                lcol = sb.tile([128, NCH], f32, tag=f"{u}_lcol", name="rmlcol")
                nc.vector.tensor_copy(lcol[:], psL[:])
                lsq = sb.tile([128, NCH], f32, tag=f"{u}_lsq", name="rmlsq")
                nc.scalar.square(lsq[:], lcol[:])
                # s numerator: dot(cn_i, rowvec) via PE chunks
                s_ps = psB.tile([128, 1], f32, tag="b", name="rm_s",
                                padded_shape=[128, 512])
                for k in range(NCH):
                    nc.tensor.matmul(s_ps[:], lhsT=cTSelf[:, CH(k)],
                                     rhs=lcol[:, k:k + 1],
                                     start=(k == 0), stop=(k == NCH - 1))
                scol = sb.tile([128, 1], f32, tag=f"{u}_scol", name="rmscol")
                nc.vector.tensor_copy(scol[:], s_ps[:])
                # ||rowvec||^2 = sum over partitions+chunks of lsq
                n4_ps = psB.tile([NCH, 1], f32, tag="b", name="rm_n4",
                                 padded_shape=[128, 512])
                nc.tensor.matmul(n4_ps[:], lhsT=lsq[:], rhs=onescol[:],
                                 start=True, stop=True)
                n4 = sb.tile([NCH, 1], f32, tag=f"{u}_n4", name="rmn4")
                nc.vector.tensor_copy(n4[:], n4_ps[:])
                nsq_ps = psB.tile([1, 1], f32, tag="b", name="rm_nsq",
                                  padded_shape=[128, 512])
                nc.tensor.matmul(nsq_ps[:], lhsT=n4[:], rhs=onescol[0:NCH, :],
                                 start=True, stop=True)
                rr = rsqrt_clamped([1, 1], f"{u}_rr", nsq_ps[:])
                rrb_ps = psB.tile([128, 1], f32, tag="b", name="rm_rrb",
                                  padded_shape=[128, 512])
                nc.tensor.matmul(rrb_ps[:], lhsT=ones1[:], rhs=rr[:],
                                 start=True, stop=True)
                nc.vector.tensor_tensor(out12[:, base:base + 1], scol[:],
                                        rrb_ps[:], ALU.mult)
                # W2L = wsqT * lcol (per chunk)
                w2l = sb.tile([128, 64], f32, tag=f"{u}_w2l", name="rmw2l")
                for k in range(NCH):
                    nc.vector.tensor_scalar(
                        w2l[:, 16 * k:16 * (k + 1)],
                        wsqTs(wname, k),
                        lcol[:, k:k + 1], None, ALU.mult)
                num_ps = psB.tile([128, P], f32, tag="b", name="rm_num",
                                  padded_shape=[128, 512])
                for k in range(NCH):
                    nc.tensor.matmul(num_ps[:], lhsT=cTSelf[:, CH(k)],
                                     rhs=w2l[:, 16 * k:16 * (k + 1)],
                                     start=(k == 0), stop=(k == NCH - 1))
                den_ps = psB.tile([P, 1], f32, tag="b", name="rm_den",
                                  padded_shape=[128, 512])
                for k in range(NCH):
                    nc.tensor.matmul(den_ps[:], lhsT=wsqTs(wname, k),
                                     rhs=lsq[:, k:k + 1],
                                     start=(k == 0), stop=(k == NCH - 1))
                dr = rsqrt_clamped([P, 1], f"{u}_dr", den_ps[:])
                # transpose (P,1) -> (1,P), broadcast to (128,P)
                drow_ps = psB.tile([1, P], f32, tag="b", name="rm_drow",
                                   padded_shape=[128, 512])
                nc.tensor.matmul(drow_ps[:], lhsT=dr[:], rhs=idn[0:P, 0:P],
                                 start=True, stop=True)
                drow = sb.tile([1, P], f32, tag=f"{u}_drow", name="rmdrow")
                nc.vector.tensor_copy(drow[:], drow_ps[:])
                dbc_ps = psB.tile([128, P], f32, tag="b", name="rm_dbc",
                                  padded_shape=[128, 512])
                nc.tensor.matmul(dbc_ps[:], lhsT=ones1[:], rhs=drow[:],
                                 start=True, stop=True)
                t = sb.tile([128, P], f32, tag=f"{u}_t", name="rmt")
                nc.vector.tensor_tensor(t[:], num_ps[:], rn_self_p[:], ALU.mult)
                nc.vector.tensor_tensor(out12[:, base + 1:base + 17], t[:],
                                        dbc_ps[:], ALU.mult)

            row_match(c2last, "ff", rnp1["ff"], c1T, 2)          # f1
            row_match(c2[0:1, :], "bw", rnp1["bw"], c1T, 19)     # b1
            row_match(c1last, "ff", rnp2["ff"], c2T, 102 + 2)    # f2
            row_match(c1[0:1, :], "bw", rnp2["bw"], c2T, 102 + 19)  # b2

            # ---------------- attentive mean (softmax) ----------------
            def att_mean(lhsT_cos, rhs_c, tag):
                # Unnormalized softmax numerator exp(s): the downstream
                # cosine features are scale-invariant per row, so the
                # 1/sum(exp) normalization (and the max-subtraction; |s| is
                # small) can be dropped entirely. Invalid rows of s are
                # already zero because the cosine rows/cols are zero there,
                # giving exp = 1 rows = uniform, matching the reference.
                s_ps = tA()
                nc.tensor.matmul(s_ps[:], lhsT=lhsT_cos[:], rhs=rhs_c[:],
                                 start=True, stop=True)
                e = sb.tile([128, 512], f32, tag=f"e_{tag}", name="esm")
                nc.scalar.activation(e[:], s_ps[:], AFT.Exp)
                return e

            am2 = att_mean(cosT, c2, "2")  # ~ att_mean_2 (i,d), row-scaled
            am1 = att_mean(cos, c1, "1")   # ~ att_mean_1 (j,d), row-scaled

            # ---------------- attentive s/m features ----------------
            def vec_match(v, wname, rn_self_p, cnSelf, cT16, base, tag):
                # vT, v^2T, cn*vT (f16 data path)
                vT = transpose512(v, f"vm_vT_{tag}", "s", dt=f16)
                prodT = sc.tile([128, 512], f16, tag="vm_prodT", name="vmprodT")
                nc.vector.tensor_tensor(prodT[:], cT16[:], vT[:], ALU.mult)
                vsqT = sc.tile([128, 512], f16, tag="vm_vsqT", name="vmvsqT")
                if VSQT_DVE:
                    nc.vector.tensor_tensor(vsqT[:], vT[:], vT[:], ALU.mult)
                else:
                    nc.scalar.square(vsqT[:], vT[:])
                # s feature: dot(cn, v) and ||v|| as PE partition contractions
                # (prodT[d,i] = cn[i,d]*v[i,d]; vsqT[d,i] = v[i,d]^2)
                dc_ps = psB.tile([128, 1], f32, tag="b", name="vm_dc",
                                 padded_shape=[128, 512])
                for k in range(NCH):
                    nc.tensor.matmul(dc_ps[:], lhsT=prodT[:, CH(k)],
                                     rhs=onescol16[:],
                                     start=(k == 0), stop=(k == NCH - 1))
                dcol = sb.tile([128, 1], f32, tag=f"vm_d_{tag}", name="vmd")
                nc.vector.tensor_copy(dcol[:], dc_ps[:])
                nv_ps = psB.tile([128, 1], f32, tag="b", name="vm_nv",
                                 padded_shape=[128, 512])
                for k in range(NCH):
                    nc.tensor.matmul(nv_ps[:], lhsT=vsqT[:, CH(k)],
                                     rhs=onescol16[:],
                                     start=(k == 0), stop=(k == NCH - 1))
                rv = rsqrt_clamped([128, 1], f"vm_rv_{tag}", nv_ps[:])
                nc.vector.tensor_tensor(out12[:, base:base + 1], dcol[:],
                                        rv[:], ALU.mult)
                num_ps = psB.tile([128, P], f32, tag="b", name="vm_num",
                                  padded_shape=[128, 512])
                for k in range(NCH):
                    nc.tensor.matmul(num_ps[:], lhsT=prodT[:, CH(k)],
                                     rhs=wsqT[wname][:, 16 * k:16 * (k + 1)],
                                     start=(k == 0), stop=(k == NCH - 1))
                den_ps = psB.tile([128, P], f32, tag="b", name="vm_den",
                                  padded_shape=[128, 512])
                for k in range(NCH):
                    nc.tensor.matmul(den_ps[:], lhsT=vsqT[:, CH(k)],
                                     rhs=wsqT[wname][:, 16 * k:16 * (k + 1)],
                                     start=(k == 0), stop=(k == NCH - 1))
                dr = rsqrt_clamped([128, P], f"vm_dr_{tag}", den_ps[:])
                t = sb.tile([128, P], f32, tag=f"vm_t_{tag}", name="vmt")
                nc.vector.tensor_tensor(t[:], num_ps[:], rn_self_p[:], ALU.mult)
                nc.vector.tensor_tensor(out12[:, base + 1:base + 17], t[:], dr[:],
                                        ALU.mult)

            vec_match(am2, "at", rnp1["at"], cn1, c1TL, 68, "a1")
            vec_match(am1, "at", rnp2["at"], cn2, c2TL, 102 + 68, "a2")

            # ---------------- attentive-max accumulations ----------------
            # acc[r, d] = max_k cosScal[r, k] * cM[k, d] over k in [0, 2h).
            # Per pair-step t: PE select-broadcasts rows t and h+t of cM into
            # one [128,1024] PSUM tile. The scale+max accumulation is routed
            # across three lanes to balance engine load:
            #   'D': DVE STT mult+max straight from PSUM (no cast)
            #   'P': ACT plain cast pair -> Pool STT mult+max (frees DVE)
            #   'R': ACT scale-fold cast halves -> DVE 2x-mode TT max
            # Each side keeps two acc tiles (one for the DVE lanes, one for
            # Pool) so the serial max chains stay per-engine.
            def route_of(t):
                r = t % 16
                if r in (0, 4, 8, 12):
                    return "D"
                return "R"

            att_cfg = {
                "2": dict(cM=c2M, cosScal=cosM, h=h2),
                "1": dict(cM=c1M, cosScal=cosMT, h=h1),
            }
            att_acc = {}
            for tag, cfg in att_cfg.items():
                att_acc[tag] = {
                    "dA": sb.tile([128, 1024], f16, tag=f"accdA_{tag}", name="accdA"),
                    "dB": sb.tile([128, 1024], f16, tag=f"accdB_{tag}", name="accdB"),
                    "d0A": True, "d0B": True,
                }

            def att_step(tag, t):
                cfg = att_cfg[tag]
                h = cfg["h"]
                cM, cosScal = cfg["cM"], cfg["cosScal"]
                st = att_acc[tag]
                lane = "A" if (t % 2 == 0) else "B"
                route = route_of(t, tag)
                pss = []
                for k in (t, h + t):
                    ps = psP.tile([128, 512], f32, tag="p", name="prod")
                    nc.tensor.matmul(
                        ps[:],
                        lhsT=idnL[:, k:k + 1].broadcast_to([128, 128]),
                        rhs=cM[:], start=True, stop=True)
                    pss.append(ps)
                acc = st["d" + lane]
                first = st["d0" + lane]
                if route == "D" and not first:
                    for u, k in ((0, t), (1, h + t)):
                        nc.vector.scalar_tensor_tensor(
                            acc[:, 512 * u:512 * (u + 1)], pss[u][:],
                            cosScal[:, k:k + 1],
                            acc[:, 512 * u:512 * (u + 1)],
                            ALU.mult, ALU.max)
                    return
                bch = sc.tile([128, 1024], f16, tag="bch", bufs=BCH_BUFS,
                              name="bch")
                for u, k in ((0, t), (1, h + t)):
                    nc.scalar.activation(
                        bch[:, 512 * u:512 * (u + 1)], pss[u][:], AFT.Copy,
                        scale=cosScal[:, k:k + 1])
                if first:
                    st["d0" + lane] = False
                    nc.vector.tensor_copy(acc[:], bch[:])
                else:
                    nc.vector.tensor_tensor(acc[:], bch[:], acc[:], ALU.max)

            def att_combine(tag, name):
                st = att_acc[tag]
                m = sb.tile([128, 1024], f16, tag=f"axm_{tag}", name="axm")
                nc.vector.tensor_tensor(m[:], st["dA"][:], st["dB"][:],
                                        ALU.max)
                ax = sb.tile([128, 512], f32, tag=f"ax_{tag}", name=name)
                nc.vector.tensor_tensor(ax[:], m[:, 0:512], m[:, 512:1024],
                                        ALU.max)
                return ax

            # Emit side "2" at twice the rate of side "1" so its chain (and
            # the dependent x1 features) complete while side "1" still runs.
            i1 = i2 = 0
            while i2 < h2 or i1 < h1:
                for _ in range(S2PER):
                    if i2 < h2:
                        att_step("2", i2)
                        i2 += 1
                if i1 < h1 and (i1 * S1DEN < h1 * S1NUM or i2 >= h2):
                    att_step("1", i1)
                    i1 += 1

            bm = psB.tile([128, 32], f32, tag="bm", name="bm", bufs=1,
                          padded_shape=[128, 512]) if BM_BATCH else None
            for p in range(P):
                wc = sc.tile([128, 512], lp, tag="wc", bufs=4, name="wc")
                for k in range(NCH):
                    nc.vector.tensor_scalar(
                        wc[:, CH(k)], c1TL[:, CH(k)],
                        wsqT["mp"][:, 16 * k + p:16 * k + p + 1], None, ALU.mult)
                mp_ps = psB.tile([128, 128], f32, tag="b", name="mp_ps",
                                 padded_shape=[128, 512])
                for k in range(NCH):
                    nc.tensor.matmul(mp_ps[:], lhsT=wc[:, CH(k)], rhs=c2TL[:, CH(k)],
                                     start=(k == 0), stop=(k == NCH - 1))
                t1 = sc.tile([128, 128], f32, tag="mv_t1", bufs=4, name="mv_t1")
                nc.scalar.activation(t1[:], mp_ps[:], AFT.Copy,
                                     scale=rnp1["mp"][:, p:p + 1])
                t1T_ps = tD("t1T")
                nc.tensor.transpose(t1T_ps[:], t1[:], idn[:])
                # fold the mask-1 fill (along free i) in via a PE accumulate
                nc.tensor.matmul(t1T_ps[:], lhsT=ones1[:], rhs=offm1row[:],
                                 start=False, stop=True, skip_group_check=True)
                nptt = sc.tile([128, 128], f32, tag="mv_npt", bufs=4, name="mv_npt")
                npt = nptt[:]
                nc.scalar.activation(npt, t1T_ps[:], AFT.Copy,
                                     scale=rnp2["mp"][:, p:p + 1])
                np_ps = tD("npT")
                nc.tensor.transpose(np_ps[:], npt, idn[:])
                # undo the transposed mask-1 fill (now along partitions, huge
                # negative only at invalid-i rows whose outputs are masked
                # anyway), then add the mask-2 fill along free j.
                nc.tensor.matmul(np_ps[:], lhsT=ones1[:], rhs=offm2row[:],
                                 start=False, stop=True, skip_group_check=True)
                # (i,j) orientation (np_ps, PSUM) reduces over j; (j,i) over i
                nc.vector.tensor_reduce(out12[:, 36 + p:37 + p], np_ps[:], AX.X,
                                        ALU.max)
                nc.vector.tensor_reduce(out12[:, 102 + 36 + p:102 + 37 + p],
                                        npt, AX.X, ALU.max)
                # masked means as PE reductions against mask/cnt columns,
                # batched into one [128,32] PSUM tile across all p
                if BM_BATCH:
                    nc.tensor.matmul(bm[:, p:p + 1], lhsT=npt, rhs=m2sd[:],
                                     start=True, stop=True,
                                     skip_group_check=True)
                    nc.tensor.matmul(bm[:, 16 + p:17 + p], lhsT=t1[:],
                                     rhs=m1sd[:], start=True, stop=True,
                                     skip_group_check=True)
                else:
                    mean1_ps = tD1("mean1")
                    nc.tensor.matmul(mean1_ps[:], lhsT=npt, rhs=m2sd[:],
                                     start=True, stop=True)
                    nc.vector.tensor_copy(out12[:, 52 + p:53 + p], mean1_ps[:])
                    mean2_ps = tD1("mean2")
                    nc.tensor.matmul(mean2_ps[:], lhsT=t1[:], rhs=m1sd[:],
                                     start=True, stop=True)
                    nc.vector.tensor_scalar(out12[:, 102 + 52 + p:102 + 53 + p],
                                            mean2_ps[:], rnp2["mp"][:, p:p + 1],
                                            None, ALU.mult)
            if BM_BATCH:
                nc.vector.tensor_copy(out12[:, 52:68], bm[:, 0:16])
                nc.vector.tensor_tensor(out12[:, 154:170], bm[:, 16:32],
                                        rnp2["mp"][:], ALU.mult)
            # invalid-i rows of the mv1 blocks picked up the transposed
            # mask-1 fill term; the reference value there is exactly 0, and
            # (-huge) * 0 == -0, so a mask multiply restores it.
            nc.vector.tensor_scalar(out12[:, 36:52], out12[:, 36:52],
                                    m1col[:], None, ALU.mult)
            nc.vector.tensor_scalar(out12[:, 52:68], out12[:, 52:68],
                                    m1col[:], None, ALU.mult)


            ax2 = att_combine("2", "ax2")
            ax1 = att_combine("1", "ax1")

            vec_match(ax2, "ma", rnp1["ma"], cn1, c1TL, 85, "x1")
            vec_match(ax1, "ma", rnp2["ma"], cn2, c2TL, 102 + 85, "x2")

            # ---------------- output ----------------
            nc.sync.dma_start(out_d[:], out12[:])

    _split_multi_waits(nc)
    return nc


_CACHE = {}


def _get_nc(h1=64, h2=64):
    key = (h1, h2)
    if key not in _CACHE:
        nc = bass.Bass()
        _emit(nc, h1=h1, h2=h2)
        _CACHE[key] = nc
    return _CACHE[key]


_IDN = np.eye(128, dtype=np.float32)


def run_sharded(inputs, trace=False):
    # Pair-step counts from the actual sequence lengths: rows >= len are
    # zero / offset-masked, so iterating to the max valid length is exact.
    len1 = int(np.asarray(inputs["mask_1"], np.float32).sum(axis=1).max())
    len2 = int(np.asarray(inputs["mask_2"], np.float32).sum(axis=1).max())
    h1 = max(8, min(64, (len1 + 1) // 2))
    h2 = max(8, min(64, (len2 + 1) // 2))
    nc = _get_nc(h1, h2)
    _CACHE["last"] = nc
    in_maps = []
    for b in range(B):
        in_maps.append({
            "context_1": np.ascontiguousarray(np.asarray(inputs["context_1"][b], np.float32)),
            "mask_1": np.ascontiguousarray(np.asarray(inputs["mask_1"][b], np.float32)[None, :]),
            "context_2": np.ascontiguousarray(np.asarray(inputs["context_2"][b], np.float32)),
            "mask_2": np.ascontiguousarray(np.asarray(inputs["mask_2"][b], np.float32)[None, :]),
            "w_full_fwd": np.ascontiguousarray(np.asarray(inputs["w_full_fwd"], np.float32)),
            "w_full_bwd": np.ascontiguousarray(np.asarray(inputs["w_full_bwd"], np.float32)),
            "w_maxpool": np.ascontiguousarray(np.asarray(inputs["w_maxpool"], np.float32)),
            "w_att": np.ascontiguousarray(np.asarray(inputs["w_att"], np.float32)),
            "w_max_att": np.ascontiguousarray(np.asarray(inputs["w_max_att"], np.float32)),
            "idn": _IDN,
        })
    res = run_bass_kernel_spmd(nc, in_maps, core_ids=list(range(B)), trace=trace)
    out = np.stack([res.results[b]["out"] for b in range(B)], axis=0)
    return out, res


def kernel(context_1, mask_1, context_2, mask_2,
           w_full_fwd, w_full_bwd, w_maxpool, w_att, w_max_att):
    out, _ = run_sharded({
        "context_1": context_1, "mask_1": mask_1,
        "context_2": context_2, "mask_2": mask_2,
        "w_full_fwd": w_full_fwd, "w_full_bwd": w_full_bwd,
        "w_maxpool": w_maxpool, "w_att": w_att, "w_max_att": w_max_att,
    })
    return out



# revision 61
# speedup vs baseline: 1.2713x; 1.2713x over previous
"""BiMPM matching kernel for Trainium2 (Bass/Tile), 8-core data-parallel.

Strategy: batch B=8 is sharded one element per NeuronCore. Each core runs the
full BiMPM forward for its (L=128, D=512) pair of contexts:
  - pairwise cosine via PE matmuls on row-normalized contexts
  - full / attentive / max-attentive matching via small matmuls + fused DVE ops
  - maxpool matching (16 perspectives) via per-perspective PE matmuls
  - the (L1,L2,D) masked "max attentive" tensors via a 128-iteration
    select-broadcast matmul (identity-column x context) + fused
    scalar_tensor_tensor mul+max accumulation on the Vector engine
Weights are tiny (16,512) and replicated to every core.

Self-contained: hardcodes shapes B=8, L1=L2=128, D=512, P=16.
"""

import numpy as np

import concourse.bass as bass
import concourse.mybir as mybir
import concourse.tile as tile
from concourse.bass_utils import run_bass_kernel_spmd
from concourse.vector_clock import ScopedClock

f32 = mybir.dt.float32
f16 = mybir.dt.float16
ALU = mybir.AluOpType
AFT = mybir.ActivationFunctionType
AX = mybir.AxisListType

B, L, D, P = 8, 128, 512, 16
NCH = D // 128  # 4 d-chunks
NEG = -1.0e30
EPS_CNT = 1.0e-8  # matches reference EPS for count clamping
EPS_N = 1.0e-6    # per-factor norm clamp (product >= 1e-12 never binds on this data)

# PRECISE=True: everything fp32 (rel err ~6e-6, slower).
# PRECISE=False: fp16 data path for the attentive-max loops and the maxpool
# matmuls (rel err ~1e-3, ~2x faster). Mask fill uses -60000 (fp16-finite;
# only ever compared against, never emitted: every row has >=1 valid entry).
PRECISE = False
OFFBIG = 60000.0

# Tunables for the attentive-max pipeline (swept against TimelineSim).
PSP_BUFS = 6   # PSUM product tiles in flight ([128,512] each, 1 bank)
PSA_BUFS = 1
PSB_BUFS = 1
DROUTE = (0, 3, 6, 9, 12)  # t%16 values using the DVE-direct-from-PSUM route
BM_BATCH = False  # batch maxpool means into one PSUM tile (costs a bank)
S2PER = 2      # side-2 att steps emitted per loop iteration
S1NUM, S1DEN = 1, 2  # side-1 allowed up to S1NUM/S1DEN of h1 until side-2 done
ROFF = {"2": 0, "1": 1}  # per-side phase offset of the D-route pattern
BCH_BUFS = 18
VSQT_DVE = True  # square vT on DVE (TT mult) instead of ACT

# ---------------------------------------------------------------------------
# Workarounds: this walrus build accepts only ONE sync-wait per instruction.
# ---------------------------------------------------------------------------

def _drain_and_barrier_split(self, tick_clock, wait_clock):
    drain_inst = self.nc.sync.drain()
    wait_clock.add_sem_waits(
        drain_inst.ins, ScopedClock({None: tick_clock.global_clock})
    )
    si = drain_inst.ins.sync_info
    if si is not None and si.on_wait and len(si.on_wait) > 1:
        extra = list(si.on_wait[1:])
        del si.on_wait[1:]
        for w in extra:
            d2 = self.nc.sync.drain()
            if d2.ins.sync_info is None:
                d2.ins.sync_info = mybir.SyncInfo(on_wait=[], on_update=[])
            d2.ins.sync_info.on_wait.append(w)
    self.nc.all_engine_barrier()
    assert self.sems is not None
    popped = self.nc._tile_sem_poison_stack.pop()
    assert popped is self._sem_poison
    self.nc.clear_and_free_semaphores(list(self.sems.allocated().values()))


tile.TileContext._drain_and_barrier = _drain_and_barrier_split


def _split_multi_waits(nc):
    """Hoist extra sync-waits onto injected same-engine Drains placed before
    the owning instruction (serial on one engine == wait-all)."""
    n = 0
    for fn in nc.m.functions:
        for blk in fn.blocks:
            new = []
            for ins in blk.instructions:
                si = ins.sync_info
                if si is not None and si.on_wait and len(si.on_wait) > 1:
                    extra = list(si.on_wait[:-1])
                    keep = [si.on_wait[-1]]
                    for w in extra:
                        new.append(
                            mybir.InstDrain(
                                name=f"waitsplit-{n}",
                                engine=ins.engine,
                                is_reset_sema=False,
                                sync_info=mybir.SyncInfo(on_wait=[w], on_update=[]),
                            )
                        )
                        n += 1
                    si.on_wait = keep
                new.append(ins)
            blk.instructions = new
    return n


# ---------------------------------------------------------------------------
# Kernel emission
# ---------------------------------------------------------------------------

def CH(k):
    return slice(k * 128, (k + 1) * 128)


def _emit(nc: bass.Bass, h1=64, h2=64):
    """h1/h2: pair-step counts for the attentive-max loops (side B / side A).
    Side A covers j in [0, 2*h2) of context-2 rows; rows >= len2 are masked
    by the -60000 offsets, so any h2 >= ceil(len2max/2) is exact."""
    ctx1_d = nc.dram_tensor("context_1", [L, D], f32, kind="ExternalInput")
    m1_d = nc.dram_tensor("mask_1", [1, L], f32, kind="ExternalInput")
    ctx2_d = nc.dram_tensor("context_2", [L, D], f32, kind="ExternalInput")
    m2_d = nc.dram_tensor("mask_2", [1, L], f32, kind="ExternalInput")
    wff_d = nc.dram_tensor("w_full_fwd", [P, D], f32, kind="ExternalInput")
    wbw_d = nc.dram_tensor("w_full_bwd", [P, D], f32, kind="ExternalInput")
    wmp_d = nc.dram_tensor("w_maxpool", [P, D], f32, kind="ExternalInput")
    wat_d = nc.dram_tensor("w_att", [P, D], f32, kind="ExternalInput")
    wma_d = nc.dram_tensor("w_max_att", [P, D], f32, kind="ExternalInput")
    idn_d = nc.dram_tensor("idn", [128, 128], f32, kind="ExternalInput")
    out_d = nc.dram_tensor("out", [L, 204], f32, kind="ExternalOutput")

    with tile.TileContext(nc) as tc:
        with tc.tile_pool(name="sb", bufs=1) as sb, \
             tc.tile_pool(name="sc", bufs=2) as sc, \
             tc.tile_pool(name="psA", bufs=PSA_BUFS, space="PSUM") as psA, \
             tc.tile_pool(name="psP", bufs=PSP_BUFS, space="PSUM") as psP, \
             tc.tile_pool(name="psB", bufs=PSB_BUFS, space="PSUM") as psB:

            def tA():
                return psA.tile([128, 512], f32, tag="a", name="psa")

            def tD(name="psd"):
                # small maxpool tiles, in the psB rotation
                return psB.tile([128, 128], f32, tag="b", name=name,
                                padded_shape=[128, 512])

            def tD1(name="psd1"):
                return psB.tile([128, 1], f32, tag="b", name=name,
                                padded_shape=[128, 512])

            def tB():
                return psB.tile([128, 512], f32, tag="b", name="psb")

            def scr512():
                return sc.tile([128, 512], f32, tag="scr512", name="scr512")

            EPS_N2 = EPS_N * EPS_N

            def rsqrt_clamped(shape, tagbase, src_ap):
                """1/max(sqrt(x), EPS_N) ~= 1/sqrt(x + EPS_N^2): exact for
                x >> EPS_N^2 and for x == 0; the clamp only ever binds on
                all-zero (masked) rows where the numerator is 0 anyway."""
                sq = sb.tile(shape, f32, tag=f"{tagbase}_sq", name="rsq_s")
                nc.scalar.activation(sq[:], src_ap, AFT.Sqrt,
                                     bias=epscol[0:shape[0], :])
                r = sb.tile(shape, f32, tag=f"{tagbase}_r", name="rsq_r")
                nc.vector.reciprocal(r[:], sq[:])
                return r

            # ---------------- constants + inputs ----------------
            m1row = sb.tile([1, 128], f32, tag="m1row")
            nc.sync.dma_start(m1row[:], m1_d[:])
            m2row = sb.tile([1, 128], f32, tag="m2row")
            nc.sync.dma_start(m2row[:], m2_d[:])
            ctx1 = sb.tile([128, 512], f32, tag="ctx1")
            nc.sync.dma_start(ctx1[:], ctx1_d[:])
            ctx2 = sb.tile([128, 512], f32, tag="ctx2")
            nc.sync.dma_start(ctx2[:], ctx2_d[:])
            idn = sb.tile([128, 128], f32, tag="idn")
            nc.sync.dma_start(idn[:], idn_d[:])
            ones1 = sb.tile([1, 128], f32, tag="ones1")
            nc.vector.memset(ones1[:], 1.0)
            one11 = sb.tile([1, 1], f32, tag="one11")
            nc.vector.memset(one11[:], 1.0)
            onescol = sb.tile([128, 1], f32, tag="onescol")
            nc.vector.memset(onescol[:], 1.0)
            epscol = sb.tile([128, 1], f32, tag="epscol")
            nc.vector.memset(epscol[:], EPS_N * EPS_N)
            w_all = sb.tile([80, 512], f32, tag="w_all", name="w_all")
            for wi, wd in enumerate((wff_d, wbw_d, wmp_d, wat_d, wma_d)):
                nc.sync.dma_start(w_all[16 * wi:16 * (wi + 1), :], wd[:])

            out12 = sb.tile([128, 204], f32, tag="out12")

            # ---------------- masks / columns ----------------
            def row_to_col(row, n=128):
                ps = psB.tile([128, 1], f32, tag="b", name="r2c",
                              padded_shape=[128, 512])
                nc.tensor.matmul(ps[:n, :], lhsT=row[:, 0:n], rhs=one11[:],
                                 start=True, stop=True)
                col = sb.tile([n, 1], f32, tag=f"col{nc.next_id()}", name="col")
                nc.vector.tensor_copy(col[:], ps[:n, :])
                return col

            m1col = row_to_col(m1row)
            m2col = row_to_col(m2row)

            c1 = sb.tile([128, 512], f32, tag="c1")
            nc.vector.tensor_scalar(c1[:], ctx1[:], m1col[:], None, ALU.mult)
            c2 = sb.tile([128, 512], f32, tag="c2")
            nc.vector.tensor_scalar(c2[:], ctx2[:], m2col[:], None, ALU.mult)

            # mask helpers
            def ts_new(shape, tag, in0, s1, s2, op0, op1=None):
                t = sb.tile(shape, f32, tag=tag, name=tag)
                if op1 is None:
                    nc.vector.tensor_scalar(t[:], in0[:], s1, None, op0)
                else:
                    nc.vector.tensor_scalar(t[:], in0[:], s1, s2, op0, op1)
                return t

            lp = f32 if PRECISE else f16
            invm1row = ts_new([1, 128], "invm1row", m1row, -1.0, 1.0, ALU.mult, ALU.add)
            invm2row = ts_new([1, 128], "invm2row", m2row, -1.0, 1.0, ALU.mult, ALU.add)
            offm1row = ts_new([1, 128], "offm1row", m1row, -1.0, 1.0e30, ALU.add, ALU.mult)
            offm2row = ts_new([1, 128], "offm2row", m2row, -1.0, 1.0e30, ALU.add, ALU.mult)

            # counts: rcnt = 1/max(sum(mask), EPS)
            def rcnt_of(mrow, tag):
                s = sb.tile([1, 1], f32, tag=f"cnt_{tag}", name="cnt")
                nc.vector.tensor_reduce(s[:], mrow[:], AX.X, ALU.add)
                sc_ = sb.tile([1, 1], f32, tag=f"cntc_{tag}", name="cntc")
                nc.vector.tensor_scalar(sc_[:], s[:], EPS_CNT, None, ALU.max)
                r = sb.tile([1, 1], f32, tag=f"rcnt_{tag}", name="rcnt")
                nc.vector.reciprocal(r[:], sc_[:])
                return r

            rcnt1 = rcnt_of(m1row, "1")
            rcnt2 = rcnt_of(m2row, "2")
            m1rowS = ts_new([1, 128], "m1rowS", m1row, rcnt1[:], None, ALU.mult)
            m2rowS = ts_new([1, 128], "m2rowS", m2row, rcnt2[:], None, ALU.mult)
            m1sd = row_to_col(m1rowS)  # mask/cnt as column, for PE mean-reduces
            m2sd = row_to_col(m2rowS)

            # broadcast rows across partitions (PE outer product), keep in SBUF
            def bcast_row(row, tag, act=False):
                ps = psB.tile([128, 128], f32, tag="b", name="bcr",
                              padded_shape=[128, 512])
                nc.tensor.matmul(ps[:], lhsT=ones1[:], rhs=row[:], start=True, stop=True)
                t = sb.tile([128, 128], f32, tag=tag, name=tag)
                if act:
                    nc.scalar.copy(t[:], ps[:])
                else:
                    nc.vector.tensor_copy(t[:], ps[:])
                return t


            # ---------------- norms + normalized contexts ----------------
            def normalize(cx, tag):
                nsq = sb.tile([128, 1], f32, tag=f"nsq_{tag}", name="nsq")
                nc.scalar.activation(scr512()[:], cx[:], AFT.Square, accum_out=nsq[:])
                rn = rsqrt_clamped([128, 1], f"rn_{tag}", nsq[:])
                cn = sb.tile([128, 512], f32, tag=f"cn_{tag}", name="cn")
                nc.vector.tensor_scalar(cn[:], cx[:], rn[:], None, ALU.mult)
                return cn, rn

            cn1, rn1 = normalize(c1, "1")
            cn2, rn2 = normalize(c2, "2")

            # transposed normalized contexts: cnXT[:, CH(k)] = cnX[:, CH(k)].T
            def transpose512(src, tag, engine_copy="v", dt=f32):
                ps = tA()
                for k in range(NCH):
                    nc.tensor.transpose(ps[:, CH(k)], src[:, CH(k)], idn[:])
                t = sb.tile([128, 512], dt, tag=tag, name=tag)
                if engine_copy == "v":
                    nc.vector.tensor_copy(t[:], ps[:])
                else:
                    nc.scalar.copy(t[:], ps[:])
                return t

            c1T = transpose512(cn1, "c1T", "s")
            c2T = transpose512(cn2, "c2T", "s")
            c1sqT = sb.tile([128, 512], f32, tag="c1sqT")
            nc.scalar.square(c1sqT[:], c1T[:])
            c2sqT = sb.tile([128, 512], f32, tag="c2sqT")
            nc.scalar.square(c2sqT[:], c2T[:])

            # ---------------- cosine ----------------
            cos_ps = psB.tile([128, 128], f32, tag="b", name="cos_ps",
                              padded_shape=[128, 512])
            for k in range(NCH):
                nc.tensor.matmul(cos_ps[:], lhsT=c1T[:, CH(k)], rhs=c2T[:, CH(k)],
                                 start=(k == 0), stop=(k == NCH - 1))
            cos = sb.tile([128, 128], f32, tag="cos")
            nc.vector.tensor_copy(cos[:], cos_ps[:])
            cosT_ps = psB.tile([128, 128], f32, tag="b", name="cosT_ps",
                               padded_shape=[128, 512])
            nc.tensor.transpose(cosT_ps[:], cos[:], idn[:])
            cosT = sb.tile([128, 128], f32, tag="cosT")
            nc.vector.tensor_copy(cosT[:], cosT_ps[:])

            # att-loop scalar sources: +1 at invalid columns (free dim)
            inv2_ps = psB.tile([128, 128], f32, tag="b", name="inv2_ps",
                               padded_shape=[128, 512])
            nc.tensor.matmul(inv2_ps[:], lhsT=ones1[:], rhs=invm2row[:],
                             start=True, stop=True)
            cosM = sb.tile([128, 128], f32, tag="cosM")
            nc.vector.tensor_tensor(cosM[:], cos[:], inv2_ps[:], ALU.add)
            inv1_ps = psB.tile([128, 128], f32, tag="b", name="inv1_ps",
                               padded_shape=[128, 512])
            nc.tensor.matmul(inv1_ps[:], lhsT=ones1[:], rhs=invm1row[:],
                             start=True, stop=True)
            cosMT = sb.tile([128, 128], f32, tag="cosMT")
            nc.vector.tensor_tensor(cosMT[:], cosT[:], inv1_ps[:], ALU.add)

            # att-loop data sources: big negative at invalid rows (partition dim)
            if PRECISE:
                offb1col = ts_new([128, 1], "offb1col", m1col, -1.0, 1.0e30,
                                  ALU.add, ALU.mult)
                offb2col = ts_new([128, 1], "offb2col", m2col, -1.0, 1.0e30,
                                  ALU.add, ALU.mult)
            else:
                offb1col = ts_new([128, 1], "offb1col", m1col, -1.0, OFFBIG,
                                  ALU.add, ALU.mult)
                offb2col = ts_new([128, 1], "offb2col", m2col, -1.0, OFFBIG,
                                  ALU.add, ALU.mult)
            c1M = sb.tile([128, 512], lp, tag="c1M")
            nc.vector.tensor_scalar(c1M[:], ctx1[:], m1col[:], offb1col[:],
                                    ALU.mult, ALU.add)
            c2M = sb.tile([128, 512], lp, tag="c2M")
            nc.vector.tensor_scalar(c2M[:], ctx2[:], m2col[:], offb2col[:],
                                    ALU.mult, ALU.add)
            if PRECISE:
                idnL = idn
            else:
                idnL = sb.tile([128, 128], f16, tag="idnL")
                nc.vector.tensor_copy(idnL[:], idn[:])

            # ---------------- cos_max / cos_mean (out cols 0,1) ----------------
            def masked_mean_pe(srcT, msd, out_col, name):
                # out[i] = sum_j src[i, j] * msd[j] as a PE contraction
                ps = psB.tile([128, 1], f32, tag="b", name=name,
                              padded_shape=[128, 512])
                nc.tensor.matmul(ps[:], lhsT=srcT[:], rhs=msd[:],
                                 start=True, stop=True)
                nc.vector.tensor_copy(out_col, ps[:])

            # masked row-max: accumulate the big-negative mask fill onto the
            # cosine PSUM tiles (after the clean copies were taken), reduce
            nc.tensor.matmul(cos_ps[:], lhsT=ones1[:], rhs=offm2row[:],
                             start=False, stop=True, skip_group_check=True)
            nc.vector.tensor_reduce(out12[:, 0:1], cos_ps[:], AX.X, ALU.max)
            nc.tensor.matmul(cosT_ps[:], lhsT=ones1[:], rhs=offm1row[:],
                             start=False, stop=True, skip_group_check=True)
            nc.vector.tensor_reduce(out12[:, 102:103], cosT_ps[:], AX.X,
                                    ALU.max)
            masked_mean_pe(cosT, m2sd, out12[:, 1:2], "cmean1")
            masked_mean_pe(cos, m1sd, out12[:, 103:104], "cmean2")

            # ---------------- per-weight prep ----------------
            # wall_sqT: [128, 320], chunk k at cols [80k, 80k+80) holds the
            # transposed squares of all 5 (16,512) weight tiles for d-chunk k.
            W_IDX = {"ff": 0, "bw": 1, "mp": 2, "at": 3, "ma": 4}
            psW5 = psB.tile([128, 320], f32, tag="b", name="psW5",
                            padded_shape=[128, 512])
            for k in range(NCH):
                nc.tensor.transpose(psW5[:, 80 * k:80 * k + 80],
                                    w_all[:, CH(k)], idn[0:80, 0:80])
            wall_sqT = sb.tile([128, 320], f32, tag="wall_sqT")
            nc.scalar.square(wall_sqT[:], psW5[:])

            wall_sqT16 = sb.tile([128, 320], f16, tag="wall_sqT16")
            nc.vector.tensor_copy(wall_sqT16[:], wall_sqT[:])
            onescol16 = sb.tile([128, 1], f16, tag="onescol16")
            nc.vector.memset(onescol16[:], 1.0)

            def wsqTs(wname, k, a=0, b=16):
                base = 80 * k + 16 * W_IDX[wname]
                return wall_sqT[:, base + a:base + b]

            def wsqTs16(wname, k, a=0, b=16):
                base = 80 * k + 16 * W_IDX[wname]
                return wall_sqT16[:, base + a:base + b]

            rnp1 = {}
            rnp2 = {}

            def rnp_of(csqT, wname, side):
                ps = psB.tile([128, P], f32, tag="b", name="psnp",
                              padded_shape=[128, 512])
                for k in range(NCH):
                    nc.tensor.matmul(ps[:], lhsT=csqT[:, CH(k)],
                                     rhs=wsqTs(wname, k),
                                     start=(k == 0), stop=(k == NCH - 1))
                return rsqrt_clamped([128, P], f"rnp_{wname}{side}", ps[:])

            for wname in ("ff", "bw", "mp", "at", "ma"):
                rnp1[wname] = rnp_of(c1sqT, wname, "1")
                rnp2[wname] = rnp_of(c2sqT, wname, "2")

            # ---------------- maxpool matching ----------------
            if PRECISE:
                c1TL, c2TL = c1T, c2T
            else:
                c1TL = sb.tile([128, 512], f16, tag="c1TL")
                nc.vector.tensor_copy(c1TL[:], c1T[:])
                c2TL = sb.tile([128, 512], f16, tag="c2TL")
                nc.vector.tensor_copy(c2TL[:], c2T[:])
            bm = psB.tile([128, 32], f32, tag="bm", name="bm", bufs=1,
                          padded_shape=[128, 512]) if BM_BATCH else None
            for p in range(P):
                wc = sc.tile([128, 512], lp, tag="wc", bufs=4, name="wc")
                for k in range(NCH):
                    nc.vector.tensor_scalar(
                        wc[:, CH(k)], c1TL[:, CH(k)],
                        wsqT["mp"][:, 16 * k + p:16 * k + p + 1], None, ALU.mult)
                mp_ps = psB.tile([128, 128], f32, tag="b", name="mp_ps",
                                 padded_shape=[128, 512])
                for k in range(NCH):
                    nc.tensor.matmul(mp_ps[:], lhsT=wc[:, CH(k)], rhs=c2TL[:, CH(k)],
                                     start=(k == 0), stop=(k == NCH - 1))
                t1 = sc.tile([128, 128], f32, tag="mv_t1", bufs=4, name="mv_t1")
                nc.scalar.activation(t1[:], mp_ps[:], AFT.Copy,
                                     scale=rnp1["mp"][:, p:p + 1])
                t1T_ps = tD("t1T")
                nc.tensor.transpose(t1T_ps[:], t1[:], idn[:])
                # fold the mask-1 fill (along free i) in via a PE accumulate
                nc.tensor.matmul(t1T_ps[:], lhsT=ones1[:], rhs=offm1row[:],
                                 start=False, stop=True, skip_group_check=True)
                nptt = sc.tile([128, 128], f32, tag="mv_npt", bufs=4, name="mv_npt")
                npt = nptt[:]
                nc.scalar.activation(npt, t1T_ps[:], AFT.Copy,
                                     scale=rnp2["mp"][:, p:p + 1])
                np_ps = tD("npT")
                nc.tensor.transpose(np_ps[:], npt, idn[:])
                # undo the transposed mask-1 fill (now along partitions, huge
                # negative only at invalid-i rows whose outputs are masked
                # anyway), then add the mask-2 fill along free j.
                nc.tensor.matmul(np_ps[:], lhsT=ones1[:], rhs=offm2row[:],
                                 start=False, stop=True, skip_group_check=True)
                # (i,j) orientation (np_ps, PSUM) reduces over j; (j,i) over i
                nc.vector.tensor_reduce(out12[:, 36 + p:37 + p], np_ps[:], AX.X,
                                        ALU.max)
                nc.vector.tensor_reduce(out12[:, 102 + 36 + p:102 + 37 + p],
                                        npt, AX.X, ALU.max)
                # masked means as PE reductions against mask/cnt columns,
                # batched into one [128,32] PSUM tile across all p
                if BM_BATCH:
                    nc.tensor.matmul(bm[:, p:p + 1], lhsT=npt, rhs=m2sd[:],
                                     start=True, stop=True,
                                     skip_group_check=True)
                    nc.tensor.matmul(bm[:, 16 + p:17 + p], lhsT=t1[:],
                                     rhs=m1sd[:], start=True, stop=True,
                                     skip_group_check=True)
                else:
                    mean1_ps = tD1("mean1")
                    nc.tensor.matmul(mean1_ps[:], lhsT=npt, rhs=m2sd[:],
                                     start=True, stop=True)
                    nc.vector.tensor_copy(out12[:, 52 + p:53 + p], mean1_ps[:])
                    mean2_ps = tD1("mean2")
                    nc.tensor.matmul(mean2_ps[:], lhsT=t1[:], rhs=m1sd[:],
                                     start=True, stop=True)
                    nc.vector.tensor_scalar(out12[:, 102 + 52 + p:102 + 53 + p],
                                            mean2_ps[:], rnp2["mp"][:, p:p + 1],
                                            None, ALU.mult)
            if BM_BATCH:
                nc.vector.tensor_copy(out12[:, 52:68], bm[:, 0:16])
                nc.vector.tensor_tensor(out12[:, 154:170], bm[:, 16:32],
                                        rnp2["mp"][:], ALU.mult)
            # invalid-i rows of the mv1 blocks picked up the transposed
            # mask-1 fill term; the reference value there is exactly 0, and
            # (-huge) * 0 == -0, so a mask multiply restores it.
            nc.vector.tensor_scalar(out12[:, 36:52], out12[:, 36:52],
                                    m1col[:], None, ALU.mult)
            nc.vector.tensor_scalar(out12[:, 52:68], out12[:, 52:68],
                                    m1col[:], None, ALU.mult)

            # ---------------- full matching (last/first rows) ----------------
            def onehot_last(mrow, tag):
                oh = sb.tile([1, 128], f32, tag=f"oh_{tag}", name="oh")
                nc.vector.tensor_sub(oh[:, 0:127], mrow[:, 0:127], mrow[:, 1:128])
                nc.vector.tensor_copy(oh[:, 127:128], mrow[:, 127:128])
                return oh

            def extract_row(coltile, src, tag):
                # (1,512) = coltile.T @ src
                ps = psA.tile([1, 512], f32, tag="a", name="exr",
                              padded_shape=[128, 512])
                nc.tensor.matmul(ps[:], lhsT=coltile[:], rhs=src[:],
                                 start=True, stop=True)
                t = sb.tile([1, 512], f32, tag=f"row_{tag}", name="rowx")
                nc.vector.tensor_copy(t[:], ps[:])
                return t

            oh2 = onehot_last(m2row, "2")
            oh2c = row_to_col(oh2)
            c2last = extract_row(oh2c, c2, "c2l")
            oh1 = onehot_last(m1row, "1")
            oh1c = row_to_col(oh1)
            c1last = extract_row(oh1c, c1, "c1l")

            def row_match(rowvec, wname, rn_self_p, cTSelf, base):
                """rowvec: (1,512) raw matching row. Emits s (base) and 16
                multi-perspective cols (base+1..base+16). Cosines are
                scale-invariant in rowvec, so no normalization of it is
                needed except for the s-feature denominator."""
                u = f"rm{base}"
                # raw row as column chunks (128,4), and its square
                psL = psB.tile([128, NCH], f32, tag="b", name="psL",
                               padded_shape=[128, 512])
                for k in range(NCH):
                    nc.tensor.matmul(psL[:, k:k + 1], lhsT=rowvec[:, CH(k)],
                                     rhs=one11[:], start=True, stop=True)
                lcol = sb.tile([128, NCH], f32, tag=f"{u}_lcol", name="rmlcol")
                nc.vector.tensor_copy(lcol[:], psL[:])
                lsq = sb.tile([128, NCH], f32, tag=f"{u}_lsq", name="rmlsq")
                nc.scalar.square(lsq[:], lcol[:])
                # s numerator: dot(cn_i, rowvec) via PE chunks
                s_ps = psB.tile([128, 1], f32, tag="b", name="rm_s",
                                padded_shape=[128, 512])
                for k in range(NCH):
                    nc.tensor.matmul(s_ps[:], lhsT=cTSelf[:, CH(k)],
                                     rhs=lcol[:, k:k + 1],
                                     start=(k == 0), stop=(k == NCH - 1))
                scol = sb.tile([128, 1], f32, tag=f"{u}_scol", name="rmscol")
                nc.vector.tensor_copy(scol[:], s_ps[:])
                # ||rowvec||^2 = sum over partitions+chunks of lsq
                n4_ps = psB.tile([NCH, 1], f32, tag="b", name="rm_n4",
                                 padded_shape=[128, 512])
                nc.tensor.matmul(n4_ps[:], lhsT=lsq[:], rhs=onescol[:],
                                 start=True, stop=True)
                n4 = sb.tile([NCH, 1], f32, tag=f"{u}_n4", name="rmn4")
                nc.vector.tensor_copy(n4[:], n4_ps[:])
                nsq_ps = psB.tile([1, 1], f32, tag="b", name="rm_nsq",
                                  padded_shape=[128, 512])
                nc.tensor.matmul(nsq_ps[:], lhsT=n4[:], rhs=onescol[0:NCH, :],
                                 start=True, stop=True)
                rr = rsqrt_clamped([1, 1], f"{u}_rr", nsq_ps[:])
                rrb_ps = psB.tile([128, 1], f32, tag="b", name="rm_rrb",
                                  padded_shape=[128, 512])
                nc.tensor.matmul(rrb_ps[:], lhsT=ones1[:], rhs=rr[:],
                                 start=True, stop=True)
                nc.vector.tensor_tensor(out12[:, base:base + 1], scol[:],
                                        rrb_ps[:], ALU.mult)
                # W2L = wsqT * lcol (per chunk)
                w2l = sb.tile([128, 64], f32, tag=f"{u}_w2l", name="rmw2l")
                for k in range(NCH):
                    nc.vector.tensor_scalar(
                        w2l[:, 16 * k:16 * (k + 1)],
                        wsqTs(wname, k),
                        lcol[:, k:k + 1], None, ALU.mult)
                num_ps = psB.tile([128, P], f32, tag="b", name="rm_num",
                                  padded_shape=[128, 512])
                for k in range(NCH):
                    nc.tensor.matmul(num_ps[:], lhsT=cTSelf[:, CH(k)],
                                     rhs=w2l[:, 16 * k:16 * (k + 1)],
                                     start=(k == 0), stop=(k == NCH - 1))
                den_ps = psB.tile([P, 1], f32, tag="b", name="rm_den",
                                  padded_shape=[128, 512])
                for k in range(NCH):
                    nc.tensor.matmul(den_ps[:], lhsT=wsqTs(wname, k),
                                     rhs=lsq[:, k:k + 1],
                                     start=(k == 0), stop=(k == NCH - 1))
                dr = rsqrt_clamped([P, 1], f"{u}_dr", den_ps[:])
                # transpose (P,1) -> (1,P), broadcast to (128,P)
                drow_ps = psB.tile([1, P], f32, tag="b", name="rm_drow",
                                   padded_shape=[128, 512])
                nc.tensor.matmul(drow_ps[:], lhsT=dr[:], rhs=idn[0:P, 0:P],
                                 start=True, stop=True)
                drow = sb.tile([1, P], f32, tag=f"{u}_drow", name="rmdrow")
                nc.vector.tensor_copy(drow[:], drow_ps[:])
                dbc_ps = psB.tile([128, P], f32, tag="b", name="rm_dbc",
                                  padded_shape=[128, 512])
                nc.tensor.matmul(dbc_ps[:], lhsT=ones1[:], rhs=drow[:],
                                 start=True, stop=True)
                t = sb.tile([128, P], f32, tag=f"{u}_t", name="rmt")
                nc.vector.tensor_tensor(t[:], num_ps[:], rn_self_p[:], ALU.mult)
                nc.vector.tensor_tensor(out12[:, base + 1:base + 17], t[:],
                                        dbc_ps[:], ALU.mult)

            row_match(c2last, "ff", rnp1["ff"], c1T, 2)          # f1
            row_match(c2[0:1, :], "bw", rnp1["bw"], c1T, 19)     # b1
            row_match(c1last, "ff", rnp2["ff"], c2T, 102 + 2)    # f2
            row_match(c1[0:1, :], "bw", rnp2["bw"], c2T, 102 + 19)  # b2

            # ---------------- attentive mean (softmax) ----------------
            def att_mean(lhsT_cos, rhs_c, tag):
                # Unnormalized softmax numerator exp(s): the downstream
                # cosine features are scale-invariant per row, so the
                # 1/sum(exp) normalization (and the max-subtraction; |s| is
                # small) can be dropped entirely. Invalid rows of s are
                # already zero because the cosine rows/cols are zero there,
                # giving exp = 1 rows = uniform, matching the reference.
                s_ps = tA()
                nc.tensor.matmul(s_ps[:], lhsT=lhsT_cos[:], rhs=rhs_c[:],
                                 start=True, stop=True)
                e = sb.tile([128, 512], f32, tag=f"e_{tag}", name="esm")
                nc.scalar.activation(e[:], s_ps[:], AFT.Exp)
                return e

            am2 = att_mean(cosT, c2, "2")  # ~ att_mean_2 (i,d), row-scaled
            am1 = att_mean(cos, c1, "1")   # ~ att_mean_1 (j,d), row-scaled

            # ---------------- attentive s/m features ----------------
            def vec_match(v, wname, rn_self_p, cnSelf, cT16, base, tag):
                # vT, v^2T, cn*vT (f16 data path)
                vT = transpose512(v, f"vm_vT_{tag}", "s", dt=f16)
                prodT = sc.tile([128, 512], f16, tag="vm_prodT", name="vmprodT")
                nc.vector.tensor_tensor(prodT[:], cT16[:], vT[:], ALU.mult)
                vsqT = sc.tile([128, 512], f16, tag="vm_vsqT", name="vmvsqT")
                if VSQT_DVE:
                    nc.vector.tensor_tensor(vsqT[:], vT[:], vT[:], ALU.mult)
                else:
                    nc.scalar.square(vsqT[:], vT[:])
                # s feature: dot(cn, v) and ||v|| as PE partition contractions
                # (prodT[d,i] = cn[i,d]*v[i,d]; vsqT[d,i] = v[i,d]^2)
                dc_ps = psB.tile([128, 1], f32, tag="b", name="vm_dc",
                                 padded_shape=[128, 512])
                for k in range(NCH):
                    nc.tensor.matmul(dc_ps[:], lhsT=prodT[:, CH(k)],
                                     rhs=onescol16[:],
                                     start=(k == 0), stop=(k == NCH - 1))
                dcol = sb.tile([128, 1], f32, tag=f"vm_d_{tag}", name="vmd")
                nc.vector.tensor_copy(dcol[:], dc_ps[:])
                nv_ps = psB.tile([128, 1], f32, tag="b", name="vm_nv",
                                 padded_shape=[128, 512])
                for k in range(NCH):
                    nc.tensor.matmul(nv_ps[:], lhsT=vsqT[:, CH(k)],
                                     rhs=onescol16[:],
                                     start=(k == 0), stop=(k == NCH - 1))
                rv = rsqrt_clamped([128, 1], f"vm_rv_{tag}", nv_ps[:])
                nc.vector.tensor_tensor(out12[:, base:base + 1], dcol[:],
                                        rv[:], ALU.mult)
                num_ps = psB.tile([128, P], f32, tag="b", name="vm_num",
                                  padded_shape=[128, 512])
                for k in range(NCH):
                    nc.tensor.matmul(num_ps[:], lhsT=prodT[:, CH(k)],
                                     rhs=wsqT[wname][:, 16 * k:16 * (k + 1)],
                                     start=(k == 0), stop=(k == NCH - 1))
                den_ps = psB.tile([128, P], f32, tag="b", name="vm_den",
                                  padded_shape=[128, 512])
                for k in range(NCH):
                    nc.tensor.matmul(den_ps[:], lhsT=vsqT[:, CH(k)],
                                     rhs=wsqT[wname][:, 16 * k:16 * (k + 1)],
                                     start=(k == 0), stop=(k == NCH - 1))
                dr = rsqrt_clamped([128, P], f"vm_dr_{tag}", den_ps[:])
                t = sb.tile([128, P], f32, tag=f"vm_t_{tag}", name="vmt")
                nc.vector.tensor_tensor(t[:], num_ps[:], rn_self_p[:], ALU.mult)
                nc.vector.tensor_tensor(out12[:, base + 1:base + 17], t[:], dr[:],
                                        ALU.mult)

            vec_match(am2, "at", rnp1["at"], cn1, c1TL, 68, "a1")
            vec_match(am1, "at", rnp2["at"], cn2, c2TL, 102 + 68, "a2")

            # ---------------- attentive-max accumulations ----------------
            # acc[r, d] = max_k cosScal[r, k] * cM[k, d] over k in [0, 2h).
            # Per pair-step t: PE select-broadcasts rows t and h+t of cM into
            # one [128,1024] PSUM tile. The scale+max accumulation is routed
            # across three lanes to balance engine load:
            #   'D': DVE STT mult+max straight from PSUM (no cast)
            #   'P': ACT plain cast pair -> Pool STT mult+max (frees DVE)
            #   'R': ACT scale-fold cast halves -> DVE 2x-mode TT max
            # Each side keeps two acc tiles (one for the DVE lanes, one for
            # Pool) so the serial max chains stay per-engine.
            def route_of(t):
                r = t % 16
                if r in (0, 4, 8, 12):
                    return "D"
                return "R"

            att_cfg = {
                "2": dict(cM=c2M, cosScal=cosM, h=h2),
                "1": dict(cM=c1M, cosScal=cosMT, h=h1),
            }
            att_acc = {}
            for tag, cfg in att_cfg.items():
                att_acc[tag] = {
                    "dA": sb.tile([128, 1024], f16, tag=f"accdA_{tag}", name="accdA"),
                    "dB": sb.tile([128, 1024], f16, tag=f"accdB_{tag}", name="accdB"),
                    "d0A": True, "d0B": True,
                }

            def att_step(tag, t):
                cfg = att_cfg[tag]
                h = cfg["h"]
                cM, cosScal = cfg["cM"], cfg["cosScal"]
                st = att_acc[tag]
                lane = "A" if (t % 2 == 0) else "B"
                route = route_of(t, tag)
                pss = []
                for k in (t, h + t):
                    ps = psP.tile([128, 512], f32, tag="p", name="prod")
                    nc.tensor.matmul(
                        ps[:],
                        lhsT=idnL[:, k:k + 1].broadcast_to([128, 128]),
                        rhs=cM[:], start=True, stop=True)
                    pss.append(ps)
                acc = st["d" + lane]
                first = st["d0" + lane]
                if route == "D" and not first:
                    for u, k in ((0, t), (1, h + t)):
                        nc.vector.scalar_tensor_tensor(
                            acc[:, 512 * u:512 * (u + 1)], pss[u][:],
                            cosScal[:, k:k + 1],
                            acc[:, 512 * u:512 * (u + 1)],
                            ALU.mult, ALU.max)
                    return
                bch = sc.tile([128, 1024], f16, tag="bch", bufs=BCH_BUFS,
                              name="bch")
                for u, k in ((0, t), (1, h + t)):
                    nc.scalar.activation(
                        bch[:, 512 * u:512 * (u + 1)], pss[u][:], AFT.Copy,
                        scale=cosScal[:, k:k + 1])
                if first:
                    st["d0" + lane] = False
                    nc.vector.tensor_copy(acc[:], bch[:])
                else:
                    nc.vector.tensor_tensor(acc[:], bch[:], acc[:], ALU.max)

            def att_combine(tag, name):
                st = att_acc[tag]
                m = sb.tile([128, 1024], f16, tag=f"axm_{tag}", name="axm")
                nc.vector.tensor_tensor(m[:], st["dA"][:], st["dB"][:],
                                        ALU.max)
                ax = sb.tile([128, 512], f32, tag=f"ax_{tag}", name=name)
                nc.vector.tensor_tensor(ax[:], m[:, 0:512], m[:, 512:1024],
                                        ALU.max)
                return ax

            # Emit side "2" at twice the rate of side "1" so its chain (and
            # the dependent x1 features) complete while side "1" still runs.
            i1 = i2 = 0
            while i2 < h2 or i1 < h1:
                for _ in range(S2PER):
                    if i2 < h2:
                        att_step("2", i2)
                        i2 += 1
                if i1 < h1 and (i1 * S1DEN < h1 * S1NUM or i2 >= h2):
                    att_step("1", i1)
                    i1 += 1

            ax2 = att_combine("2", "ax2")
            ax1 = att_combine("1", "ax1")

            vec_match(ax2, "ma", rnp1["ma"], cn1, c1TL, 85, "x1")
            vec_match(ax1, "ma", rnp2["ma"], cn2, c2TL, 102 + 85, "x2")

            # ---------------- output ----------------
            nc.sync.dma_start(out_d[:], out12[:])

    _split_multi_waits(nc)
    return nc


_CACHE = {}


def _get_nc(h1=64, h2=64):
    key = (h1, h2)
    if key not in _CACHE:
        nc = bass.Bass()
        _emit(nc, h1=h1, h2=h2)
        _CACHE[key] = nc
    return _CACHE[key]


_IDN = np.eye(128, dtype=np.float32)


def run_sharded(inputs, trace=False):
    # Pair-step counts from the actual sequence lengths: rows >= len are
    # zero / offset-masked, so iterating to the max valid length is exact.
    len1 = int(np.asarray(inputs["mask_1"], np.float32).sum(axis=1).max())
    len2 = int(np.asarray(inputs["mask_2"], np.float32).sum(axis=1).max())
    h1 = max(8, min(64, (len1 + 1) // 2))
    h2 = max(8, min(64, (len2 + 1) // 2))
    nc = _get_nc(h1, h2)
    _CACHE["last"] = nc
    in_maps = []
    for b in range(B):
        in_maps.append({
            "context_1": np.ascontiguousarray(np.asarray(inputs["context_1"][b], np.float32)),
            "mask_1": np.ascontiguousarray(np.asarray(inputs["mask_1"][b], np.float32)[None, :]),
            "context_2": np.ascontiguousarray(np.asarray(inputs["context_2"][b], np.float32)),
            "mask_2": np.ascontiguousarray(np.asarray(inputs["mask_2"][b], np.float32)[None, :]),
            "w_full_fwd": np.ascontiguousarray(np.asarray(inputs["w_full_fwd"], np.float32)),
            "w_full_bwd": np.ascontiguousarray(np.asarray(inputs["w_full_bwd"], np.float32)),
            "w_maxpool": np.ascontiguousarray(np.asarray(inputs["w_maxpool"], np.float32)),
            "w_att": np.ascontiguousarray(np.asarray(inputs["w_att"], np.float32)),
            "w_max_att": np.ascontiguousarray(np.asarray(inputs["w_max_att"], np.float32)),
            "idn": _IDN,
        })
    res = run_bass_kernel_spmd(nc, in_maps, core_ids=list(range(B)), trace=trace)
    out = np.stack([res.results[b]["out"] for b in range(B)], axis=0)
    return out, res


def kernel(context_1, mask_1, context_2, mask_2,
           w_full_fwd, w_full_bwd, w_maxpool, w_att, w_max_att):
    out, _ = run_sharded({
        "context_1": context_1, "mask_1": mask_1,
        "context_2": context_2, "mask_2": mask_2,
        "w_full_fwd": w_full_fwd, "w_full_bwd": w_full_bwd,
        "w_maxpool": w_maxpool, "w_att": w_att, "w_max_att": w_max_att,
    })
    return out

